# revision 19
# baseline (speedup 1.0000x reference)
"""BNN VGG7 forward pass on 8 Trainium2 NeuronCores (pure data parallel).

Network (reference.py): 6x [conv3x3(replicate pad, binarized w) -> BN ->
hardtanh -> binarize] with maxpool2 after blocks 1,3,5, then
fc1(8192->1024) -> BN -> binarize -> fc2(1024->10) -> *scale.

Key exactness property: weights and post-block-0 activations are exactly
+-1, so conv/fc sums are small integers computed exactly in fp32 PSUM even
with bf16 operands, and each BN+hardtanh+binarize collapses to a
per-channel threshold compare y >= t with t = m - b/(g/sqrt(v+eps))
(computed in float64 on host). Only block 0 (real-valued input) runs in
fp32 on the PE.

Sharding: batch 256 -> 32 per core, weights replicated. Forward only, no
collectives; host gathers the 8x[32,10] outputs.
"""

import json

import numpy as np
import ml_dtypes
from contextlib import ExitStack

import concourse.bass as bass
import concourse.mybir as mybir
import concourse.tile as tile
import concourse.bass_utils as bass_utils
import concourse.bass2jax as bass2jax
from concourse.bass_utils import run_bass_kernel_spmd


# --------------------------------------------------------------------------
# Walrus in this toolchain rejects instructions carrying more than one sync
# wait ("Too many sync wait commands", CoreV*GenImpl setupSyncWait). Tile's
# sem assigner freely attaches several waits to one instruction. Legalize the
# BIR right before compilation: hoist all-but-one wait of any instruction
# onto standalone EventSemaphore (wait-only) instructions on the same engine
# immediately before it — identical blocking semantics, one wait per slot.
# --------------------------------------------------------------------------

def _legalize_bir_json(bir_json):
    m = json.loads(bir_json)
    changed = 0
    for fn in m.get("functions", []):
        for blk in fn.get("blocks", []):
            out = []
            for inst in blk.get("instructions", []):
                si = inst.get("sync_info") or {}
                waits = si.get("on_wait") or []
                if len(waits) > 1 and inst.get("opcode") != "EventSemaphore":
                    keep = waits[-1]
                    for i, w in enumerate(waits[:-1]):
                        out.append({
                            "debug": inst.get("debug", 0),
                            "engine": inst["engine"],
                            "ins": [],
                            "name": f"{inst['name']}_hw{i}",
                            "opcode": "EventSemaphore",
                            "outs": [],
                            "sync_info": {"on_update": [], "on_wait": [w]},
                        })
                    si["on_wait"] = [keep]
                    inst["sync_info"] = si
                    changed += 1
                out.append(inst)
            blk["instructions"] = out
    if changed:
        print(f"bir legalizer: split waits on {changed} instructions", flush=True)
    return json.dumps(m).encode()


_ORIG_COMPILE = bass_utils.compile_bir_kernel


def _patched_compile_bir_kernel(bir_json, tmpdir, neff_name="file.neff"):
    return _ORIG_COMPILE(_legalize_bir_json(bir_json), tmpdir, neff_name=neff_name)


if getattr(bass_utils.compile_bir_kernel, "__name__", "") != "_patched_compile_bir_kernel":
    bass_utils.compile_bir_kernel = _patched_compile_bir_kernel
    bass2jax.compile_bir_kernel = _patched_compile_bir_kernel

F32 = mybir.dt.float32
BF16 = mybir.dt.bfloat16
AF = mybir.ActivationFunctionType
BF16NP = ml_dtypes.bfloat16

EPS = 1e-5
NCORES = 8
B = 32  # samples per core

# (cin, cout, H, pool?) per conv block
CONV_CFG = [
    (3, 128, 32, False),
    (128, 128, 32, True),
    (128, 256, 16, False),
    (256, 256, 16, True),
    (256, 512, 8, False),
    (512, 512, 8, True),
]
# negt column offsets per layer in the packed [128, 14] threshold tensor
NEGT_COL = [0, 1, 2, 4, 6, 10]


def _s(dy, dx):
    return dy * 3 + dx


def build_nc():
    nc = bass.Bass(trn_type="TRN2")

    # ---- DRAM I/O ----
    # L0 input: im2col (K=27) tri-split into exact bf16 hi/mid/lo -> K=81
    im81_d = nc.dram_tensor("im2col81", [81, 32768], BF16, kind="ExternalInput")
    w0_d = nc.dram_tensor("w081", [81, 128], BF16, kind="ExternalInput")
    w1_d = nc.dram_tensor("w1t", [128, 9 * 128], BF16, kind="ExternalInput")
    w2_d = nc.dram_tensor("w2t", [128, 9 * 256], BF16, kind="ExternalInput")
    w3_d = nc.dram_tensor("w3t", [2, 128, 9 * 256], BF16, kind="ExternalInput")
    w4_d = nc.dram_tensor("w4t", [2, 128, 9 * 512], BF16, kind="ExternalInput")
    w5_d = nc.dram_tensor("w5t", [4, 128, 9 * 512], BF16, kind="ExternalInput")
    negt_d = nc.dram_tensor("negt", [128, 14], F32, kind="ExternalInput")
    w1p_d = nc.dram_tensor("w1pt", [64, 128, 1024], BF16, kind="ExternalInput")
    negt6_d = nc.dram_tensor("negt6r", [32, 1024], F32, kind="ExternalInput")
    w2f_d = nc.dram_tensor("w2fct", [128, 80], BF16, kind="ExternalInput")
    id_d = nc.dram_tensor("ident32", [32, 32], BF16, kind="ExternalInput")
    out_d = nc.dram_tensor("out", [32, 10], F32, kind="ExternalOutput")

    with tile.TileContext(nc) as tc:
        with ExitStack() as main:
            cpool = main.enter_context(tc.tile_pool(name="consts", bufs=1))

            def const_tile(shape, dtype, dram, name):
                t = cpool.tile(shape, dtype, name=name, tag=name)
                nc.sync.dma_start(out=t[:], in_=dram[:])
                return t

            negt_sb = const_tile([128, 14], F32, negt_d, "negt_sb")
            w0_sb = const_tile([81, 128], BF16, w0_d, "w0_sb")
            wt_sb = {
                1: [const_tile([128, 9 * 128], BF16, w1_d, "wt1")],
                2: [const_tile([128, 9 * 256], BF16, w2_d, "wt2")],
                3: [const_tile([128, 9 * 256], BF16, w3_d[i], f"wt3_{i}") for i in range(2)],
                4: [const_tile([128, 9 * 512], BF16, w4_d[i], f"wt4_{i}") for i in range(2)],
                5: [const_tile([128, 9 * 512], BF16, w5_d[i], f"wt5_{i}") for i in range(4)],
            }
            negt6_sb = const_tile([32, 1024], F32, negt6_d, "negt6_sb")
            w2f_sb = const_tile([128, 80], BF16, w2f_d, "w2f_sb")
            id_sb = const_tile([32, 32], BF16, id_d, "id_sb")

            h6_sb = [cpool.tile([128, 512], BF16, name=f"h6_{i}", tag=f"h6_{i}")
                     for i in range(4)]

            stg = main.enter_context(tc.tile_pool(name="stg", bufs=8))

            def hview(t, nb, hp):
                return t[:].rearrange("p (b h w) -> p b h w", b=nb, h=hp, w=hp)

            def pad_fill(t, nb, hp):
                v = hview(t, nb, hp)
                # cols first (interior rows), then rows (full width incl corners)
                nc.vector.tensor_copy(out=v[:, :, 1:hp - 1, 0], in_=v[:, :, 1:hp - 1, 1])
                nc.vector.tensor_copy(out=v[:, :, 1:hp - 1, hp - 1], in_=v[:, :, 1:hp - 1, hp - 2])
                nc.vector.tensor_copy(out=v[:, :, 0, :], in_=v[:, :, 1, :])
                nc.vector.tensor_copy(out=v[:, :, hp - 1, :], in_=v[:, :, hp - 2, :])

            # ================= L0: fp32, K=27 im2col, 4x row-packed =================
            es_h1 = ExitStack()
            ph1 = es_h1.enter_context(tc.tile_pool(name="ph1", bufs=1, side="right"))
            h1p = ph1.tile([128, 32 * 34 * 34], BF16, name="h1p", tag="h1p")
            h1v = hview(h1p, B, 34)

            with ExitStack() as es0:
                p0 = es0.enter_context(tc.tile_pool(name="p0", bufs=2, side="right"))
                ps0 = es0.enter_context(tc.tile_pool(name="ps0", bufs=2, space="PSUM"))
                for c in range(4):
                    im_sb = p0.tile([81, 8192], BF16, name=f"im81_{c}", tag="im81")
                    nc.sync.dma_start(out=im_sb[:], in_=im81_d[:, c * 8192:(c + 1) * 8192])
                    for g in range(4):
                        pt = ps0.tile([128, 2048], F32, name=f"pt0_{c}_{g}", tag="pt0")
                        for k in range(4):
                            nc.tensor.matmul(
                                pt[:, k * 512:(k + 1) * 512],
                                w0_sb[:],
                                im_sb[:, (g * 4 + k) * 512:(g * 4 + k + 1) * 512],
                                start=True, stop=True,
                            )
                        b0 = 8 * c + 2 * g
                        nc.scalar.activation(
                            out=h1v[:, b0:b0 + 2, 1:33, 1:33],
                            in_=pt[:].rearrange("p (b h w) -> p b h w", b=2, h=32, w=32),
                            func=AF.Sign,
                            bias=negt_sb[:, 0:1],
                        )
                pad_fill(h1p, B, 34)

            # ================= conv blocks 1..5 =================
            def conv_layer(li, hin_tiles, hin_nb, hin_hp, hout_writer):
                """hin_tiles: list of [128, nb*hp*hp] bf16 chunk tiles.
                hout_writer(co, T, produce): produce(out_view_shape_fn) ...
                Instead: returns via closure below."""

            es_prev = es_h1
            hcur = [h1p]
            cur_nb, cur_hp = B, 34

            es_ps = ExitStack()
            ps = es_ps.enter_context(tc.tile_pool(name="ps", bufs=8, space="PSUM"))

            for li in range(1, 6):
                cin, cout, H, do_pool = CONV_CFG[li]
                cin_ch = cin // 128
                cout_ch = cout // 128
                hp = H + 2
                # tile geometry: nb samples per tile so that nb*H*H == 512
                # (L1: half-sample tiles, 16 rows x 32 cols)
                nb = max(512 // (H * H), 1)
                ntiles = (B * H * H) // 512
                hin_v = [hview(t, cur_nb, cur_hp) for t in hcur]

                # output tensor(s)
                if li == 5:
                    # pooled output goes straight into h6 (no padding)
                    hout = h6_sb
                    out_nb, out_hp = B, 4
                    hout_v = [t[:].rearrange("p (b h w) -> p b h w", b=B, h=4, w=4)
                              for t in hout]
                else:
                    Hn = H // 2 if do_pool else H
                    out_hp = Hn + 2
                    es_next = ExitStack()
                    # alternate SBUF sides so pool releases stay LIFO per side
                    pn = es_next.enter_context(tc.tile_pool(
                        name=f"ph{li + 1}", bufs=1,
                        side="right" if li % 2 == 0 else "left"))
                    hout = [pn.tile([128, B * out_hp * out_hp], BF16,
                                    name=f"h{li + 1}p_{i}", tag=f"h{li + 1}p_{i}")
                            for i in range(cout_ch)]
                    hout_v = [hview(t, B, out_hp) for t in hout]

                for co in range(cout_ch):
                    bias = negt_sb[:, NEGT_COL[li] + co: NEGT_COL[li] + co + 1]
                    for T in range(ntiles):
                        b0 = nb * T
                        pt = ps.tile([128, 512], F32, name=f"pt{li}_{co}_{T}", tag="pt")
                        n_acc = cin_ch * 9
                        i_acc = 0
                        for ci in range(cin_ch):
                            for s in range(9):
                                dy, dx = s // 3, s % 3
                                if li == 1:
                                    b = T // 2
                                    r0 = 16 * (T % 2)
                                    rhs = hin_v[ci][:, b, r0 + dy:r0 + dy + 16, dx:dx + 32]
                                else:
                                    rhs = hin_v[ci][:, b0:b0 + nb, dy:dy + H, dx:dx + H]
                                nc.tensor.matmul(
                                    pt[:],
                                    wt_sb[li][ci][:, s * cout + co * 128: s * cout + co * 128 + 128],
                                    rhs,
                                    start=(i_acc == 0), stop=(i_acc == n_acc - 1),
                                )
                                i_acc += 1
                        # ---- evacuate ----
                        if not do_pool:
                            if li == 1:
                                raise AssertionError
                            ov = hout_v[co][:, b0:b0 + nb, 1:H + 1, 1:H + 1]
                            nc.scalar.activation(
                                out=ov,
                                in_=pt[:].rearrange("p (b h w) -> p b h w", b=nb, h=H, w=H),
                                func=AF.Sign, bias=bias,
                            )
                        elif li == 1:
                            # tile = 16 rows x 32 cols of sample b
                            stage = stg.tile([128, 512], BF16, name=f"st{li}_{co}_{T}", tag="stage")
                            nc.scalar.activation(out=stage[:], in_=pt[:], func=AF.Sign, bias=bias)
                            sv = stage[:].rearrange("p (h w2 two) -> p h w2 two", h=16, two=2)
                            tmp1 = stg.tile([128, 256], BF16, name=f"tm{li}_{co}_{T}", tag="tmp1")
                            t1o = tmp1[:].rearrange("p (h w2) -> p h w2", h=16, w2=16)
                            nc.vector.tensor_max(out=t1o, in0=sv[:, :, :, 0], in1=sv[:, :, :, 1])
                            t1v = tmp1[:].rearrange("p (h2 two w2) -> p h2 two w2", two=2, w2=16)
                            b = T // 2
                            r0p = 1 + 8 * (T % 2)
                            ov = hout_v[co][:, b, r0p:r0p + 8, 1:17]
                            nc.vector.tensor_max(out=ov, in0=t1v[:, :, 0, :], in1=t1v[:, :, 1, :])
                        else:
                            stage = stg.tile([128, 512], BF16, name=f"st{li}_{co}_{T}", tag="stage")
                            nc.scalar.activation(out=stage[:], in_=pt[:], func=AF.Sign, bias=bias)
                            sv = stage[:].rearrange(
                                "p (b h w2 two) -> p b h w2 two", b=nb, h=H, two=2)
                            tmp1 = stg.tile([128, 256], BF16, name=f"tm{li}_{co}_{T}", tag="tmp1")
                            t1o = tmp1[:].rearrange(
                                "p (b h w2) -> p b h w2", b=nb, h=H, w2=H // 2)
                            nc.vector.tensor_max(out=t1o, in0=sv[:, :, :, :, 0], in1=sv[:, :, :, :, 1])
                            t1v = tmp1[:].rearrange(
                                "p (b h2 two w2) -> p b h2 two w2", b=nb, two=2, w2=H // 2)
                            if li == 5:
                                ov = hout_v[co][:, b0:b0 + nb, :, :]
                            else:
                                ov = hout_v[co][:, b0:b0 + nb, 1:H // 2 + 1, 1:H // 2 + 1]
                            nc.vector.tensor_max(out=ov, in0=t1v[:, :, :, 0, :], in1=t1v[:, :, :, 1, :])

                if li != 5:
                    for t in hout:
                        pad_fill(t, B, out_hp)

                es_prev.close()
                es_prev = ExitStack() if li == 5 else es_next
                hcur = hout
                cur_nb, cur_hp = B, out_hp

            # ================= FC head =================
            es_ps.close()
            with ExitStack() as esf:
                fps = esf.enter_context(tc.tile_pool(name="fps", bufs=1, space="PSUM"))
                fps2 = esf.enter_context(tc.tile_pool(name="fps2", bufs=2, space="PSUM"))
                fpool = esf.enter_context(tc.tile_pool(name="fpool", bufs=1))
                w1pool = esf.enter_context(tc.tile_pool(name="w1pool", bufs=16))

                fc1_ps = fps.tile([32, 1024], F32, name="fc1_ps", tag="fc1_ps")
                h6v = [t[:].rearrange("p (b s) -> p b s", s=16) for t in h6_sb]
                for j in range(64):
                    s, cj = j // 4, j % 4
                    wj = w1pool.tile([128, 1024], BF16, name=f"w1c_{j}", tag="w1c")
                    nc.sync.dma_start(out=wj[:], in_=w1p_d[j])
                    lhsT = h6v[cj][:, :, s]
                    for half in range(2):
                        nc.tensor.matmul(
                            fc1_ps[:, half * 512:(half + 1) * 512],
                            lhsT,
                            wj[:, half * 512:(half + 1) * 512],
                            start=(j == 0), stop=(j == 63),
                        )

                d_sb = fpool.tile([32, 1024], F32, name="d_sb", tag="d_sb")
                nc.vector.tensor_add(out=d_sb[:], in0=fc1_ps[:], in1=negt6_sb[:])
                h7 = fpool.tile([32, 1024], BF16, name="h7", tag="h7")
                nc.scalar.activation(out=h7[:], in_=d_sb[:], func=AF.Sign)

                h7t = []
                for jj in range(8):
                    ptr = fps2.tile([128, 32], F32, name=f"ptr_{jj}", tag="ptr")
                    nc.tensor.matmul(ptr[:], h7[:, jj * 128:(jj + 1) * 128], id_sb[:],
                                     start=True, stop=True)
                    ht = fpool.tile([128, 32], BF16, name=f"h7t_{jj}", tag=f"h7t_{jj}")
                    nc.scalar.activation(out=ht[:], in_=ptr[:], func=AF.Copy)
                    h7t.append(ht)

                ps2 = fps2.tile([32, 10], F32, name="ps2", tag="ps2")
                for jj in range(8):
                    nc.tensor.matmul(ps2[:], h7t[jj][:], w2f_sb[:, jj * 10:(jj + 1) * 10],
                                     start=(jj == 0), stop=(jj == 7))
                out_sb = fpool.tile([32, 10], F32, name="out_sb", tag="out_sb")
                nc.scalar.activation(out=out_sb[:], in_=ps2[:], func=AF.Copy)
                nc.sync.dma_start(out=out_d[:], in_=out_sb[:])

            es_prev.close()

    return nc


# ======================= host-side preparation =======================

def _sgn(w):
    w = np.asarray(w, np.float32)
    return np.where(w >= 0, np.float32(1.0), np.float32(-1.0))


def _neg_thresh(bn):
    g, b, m, v = [np.asarray(a, np.float64) for a in bn]
    s = g / np.sqrt(v + EPS)
    return (-(m - b / s)).astype(np.float32)


def _conv_wt(w):
    # [CO,CI,3,3] -> chunks [CI/128, 128, 9*CO] with free index s*CO + co
    w = _sgn(w)
    co, ci = w.shape[0], w.shape[1]
    arr = w.transpose(1, 2, 3, 0).reshape(ci, 9 * co)  # [ci, (s, co)]
    n = max(ci // 128, 1)
    if ci % 128 == 0:
        arr = arr.reshape(n, 128, 9 * co)
    else:
        arr = arr[None]
    return np.ascontiguousarray(arr.astype(BF16NP))


def _prep_shared(conv_ws, fc_ws, bns):
    d = {}
    # L0 weights: [27, 128] repeated 3x (for the hi/mid/lo input split)
    w0 = _sgn(conv_ws[0])  # [128, 3, 3, 3]
    w0t = w0.transpose(1, 2, 3, 0).reshape(27, 128)
    d["w081"] = np.ascontiguousarray(
        np.concatenate([w0t, w0t, w0t], axis=0).astype(BF16NP))

    wt1 = _conv_wt(conv_ws[1]); d["w1t"] = wt1[0]
    wt2 = _conv_wt(conv_ws[2]); d["w2t"] = wt2[0]
    d["w3t"] = _conv_wt(conv_ws[3])
    d["w4t"] = _conv_wt(conv_ws[4])
    d["w5t"] = _conv_wt(conv_ws[5])

    negt = np.zeros((128, 14), np.float32)
    for li in range(6):
        nt = _neg_thresh(bns[li])  # [cout]
        cout = nt.shape[0]
        for co in range(max(cout // 128, 1)):
            negt[:, NEGT_COL[li] + co] = nt[co * 128:(co + 1) * 128]
    d["negt"] = negt

    # FC1: k' = s*512 + c permutation
    w1s = _sgn(fc_ws[0])  # [1024, 8192]
    w1r = w1s.reshape(1024, 512, 16)  # [co, c, s]
    w1p = w1r.transpose(2, 1, 0).reshape(8192, 1024)  # [k', co]
    d["w1pt"] = np.ascontiguousarray(w1p.reshape(64, 128, 1024).astype(BF16NP))

    nt6 = _neg_thresh(bns[6])  # [1024]
    d["negt6r"] = np.broadcast_to(nt6[None, :], (32, 1024)).copy()

    w2s = _sgn(fc_ws[1])  # [10, 1024]
    w2t = w2s.T.reshape(8, 128, 10).transpose(1, 0, 2).reshape(128, 80)
    d["w2fct"] = np.ascontiguousarray(w2t.astype(BF16NP))

    d["ident32"] = np.eye(32, dtype=BF16NP)
    return d


def _im2col81(xc):
    """xc: [32, 3, 32, 32] fp32 -> [81, 32768] bf16 tri-split im2col.

    Any fp32 value equals hi+mid+lo with hi/mid/lo bf16 (up to ~2^-27 rel
    residual in rare carry cases), so the K=81 bf16 matmul reproduces the
    fp32 K=27 conv essentially exactly (accumulation is fp32 in PSUM).
    """
    xp = np.pad(np.asarray(xc, np.float32), ((0, 0), (0, 0), (1, 1), (1, 1)), mode="edge")
    cols = np.empty((27, 32 * 1024), np.float32)
    k = 0
    for ci in range(3):
        for dy in range(3):
            for dx in range(3):
                cols[k] = xp[:, ci, dy:dy + 32, dx:dx + 32].reshape(-1)
                k += 1
    hi = cols.astype(BF16NP)
    mid = (cols - hi.astype(np.float32)).astype(BF16NP)
    lo = (cols - hi.astype(np.float32) - mid.astype(np.float32)).astype(BF16NP)
    return np.ascontiguousarray(np.concatenate([hi, mid, lo], axis=0))


_CACHE = {}


def kernel(x, conv_ws, fc_ws, bns, scale):
    x = np.asarray(x, np.float32)
    if "nc" not in _CACHE:
        _CACHE["nc"] = build_nc()
    nc = _CACHE["nc"]

    shared = _prep_shared(conv_ws, fc_ws, bns)
    in_maps = []
    for c in range(NCORES):
        m = dict(shared)
        m["im2col81"] = _im2col81(x[c * B:(c + 1) * B])
        in_maps.append(m)

    res = run_bass_kernel_spmd(nc, in_maps, list(range(NCORES)))
    out = np.concatenate([np.asarray(r["out"]) for r in res.results], axis=0)
    return (out * np.float32(np.asarray(scale))).astype(np.float32)


def profile_once(inputs):
    """Run once with NTFF tracing; returns HW exec time in ns (or None)."""
    x = np.asarray(inputs["x"], np.float32)
    if "nc" not in _CACHE:
        _CACHE["nc"] = build_nc()
    nc = _CACHE["nc"]
    shared = _prep_shared(inputs["conv_ws"], inputs["fc_ws"], inputs["bns"])
    in_maps = []
    for c in range(NCORES):
        m = dict(shared)
        m["im2col81"] = _im2col81(x[c * B:(c + 1) * B])
        in_maps.append(m)
    import tempfile
    tmpdir = tempfile.mkdtemp(prefix="bnnprof_")
    try:
        res = run_bass_kernel_spmd(nc, in_maps, list(range(NCORES)), trace=True,
                                   tmpdir=tmpdir)
        print("profile artifacts in:", tmpdir)
        return res.exec_time_ns
    except Exception as e:  # profiling is best-effort
        print("profile failed:", e)
        return None
